# revision 5
# baseline (speedup 1.0000x reference)
"""Trainium2 Bass kernel for a 2-layer multi-head GAT encoder + heads.

Row-shards the 4096x4096 masked attention across 8 NeuronCores (512 query
rows each), keeps the NxN matrices fused in SBUF (never materialized in
HBM), works in a transposed [feature, node] layout so every contraction
lands on the tensor engine, and AllGathers node features between layers.
"""

import numpy as np
import ml_dtypes

import concourse.bass as bass
import concourse.mybir as mybir
import concourse.tile as tile
from concourse import bacc
from concourse.bass_utils import run_bass_kernel_spmd
from concourse.bass_interp import get_hw_module

F32 = mybir.dt.float32
F32R = mybir.dt.float32r
FP8 = mybir.dt.float8e5
I16 = mybir.dt.int16
AF = mybir.ActivationFunctionType
OP = mybir.AluOpType

N, IN_DIM, H1, H2, HEADS, DEC = 4096, 512, 256, 128, 4, 512
W = 8                 # cores
I = N // W            # queries per core (512)
NJ = N // 128         # 32 j-chunks
NEGDEV = -512.0       # mask add; post-lrelu -102.4, exp() underflows to 0
WQ1 = H1 + 4          # [W1 | qd | pad] -> 260
WQ2 = 2 * H2          # [W2 | qd | pad] -> 256 (fp32r full speed needs >=256)


def _build():
    nc = bacc.Bacc("TRN2", target_bir_lowering=False, debug=False, num_devices=W)

    def inp(name, shape, dt):
        return nc.dram_tensor(name, list(shape), dt, kind="ExternalInput")

    xoT = inp("xoT", [IN_DIM, N], F32R)
    xaT = inp("xaT", [IN_DIM, N], F32R)
    hmyT1_o = inp("hmyT1_o", [IN_DIM, I], F32R)
    hmyT1_a = inp("hmyT1_a", [IN_DIM, I], F32R)
    nm = inp("nm", [N, I], FP8)
    wq1 = inp("wq1", [HEADS, IN_DIM, WQ1], F32R)
    qs1 = inp("qs1", [HEADS, IN_DIM, 1], F32R)
    wq2 = inp("wq2", [HEADS, H1, WQ2], F32R)
    qs2 = inp("qs2", [HEADS, H1, 1], F32R)
    lin1 = inp("lin1", [HEADS * H1, H1], F32R)
    res1 = inp("res1", [IN_DIM, H1], F32R)
    lin2 = inp("lin2", [HEADS * H2, H2], F32R)
    res2 = inp("res2", [H1, H2], F32R)
    identf8 = inp("identf8", [128, 128], FP8)
    identr = inp("identr", [128, 128], F32R)
    onesr = inp("onesr", [128, 512], F32R)
    mlp1_w = inp("mlp1_w", [H2, H2], F32)
    mlp1_b = inp("mlp1_b", [1, H2], F32)
    disc_wT = inp("disc_wT", [H2, H2], F32)
    disc_b = inp("disc_b", [1, 1], F32)
    dec1_w = inp("dec1_w", [2 * H2, DEC], F32)
    dec1_bw = inp("dec1_bw", [128, DEC // 128], F32)
    dec2_w = inp("dec2_w", [DEC, 1], F32)
    dec2_b = inp("dec2_b", [1, 1], F32)
    aw1 = inp("aw1", [H2, 1], F32)
    advb_s = inp("advb_s", [1, 1], F32)
    idx1w = inp("idx1w", [128, I // 16], I16)
    idx2w = inp("idx2w", [128, I // 16], I16)

    x2slice_d = nc.dram_tensor("x2slice", [I, H2], F32, kind="ExternalOutput")
    disc4_d = nc.dram_tensor("disc4", [4, N], F32, kind="ExternalOutput")
    advrows_d = nc.dram_tensor("advrows", [2, N], F32, kind="ExternalOutput")
    log1s_d = nc.dram_tensor("log1s", [1, I], F32, kind="ExternalOutput")
    logs_d = nc.dram_tensor("logs", [1, I], F32, kind="ExternalOutput")

    ag1_in, ag1_out, ag2_in, ag2_out = {}, {}, {}, {}
    for e in ("o", "a"):
        ag1_in[e] = nc.dram_tensor(f"ag1_in_{e}", [H1, I], F32R)
        ag1_out[e] = nc.dram_tensor(f"ag1_out_{e}", [W * H1, I], F32R, addr_space="Shared")
        ag2_in[e] = nc.dram_tensor(f"ag2_in_{e}", [H2, I], F32R)
        ag2_out[e] = nc.dram_tensor(f"ag2_out_{e}", [W * H2, I], F32R, addr_space="Shared")
    agn_in = nc.dram_tensor("agn_in", [I, H2], F32)
    agn_out = nc.dram_tensor("agn_out", [N, H2], F32, addr_space="Shared")
    agn_local = nc.dram_tensor("agn_local", [N, H2], F32)
    RG = [list(range(W))]

    with tile.TileContext(nc) as tc:
        import contextlib
        ctx = contextlib.ExitStack()
        cn = ctx.enter_context(tc.tile_pool(name="cn", bufs=1))
        big = ctx.enter_context(tc.tile_pool(name="big", bufs=1))
        wk = ctx.enter_context(tc.tile_pool(name="wk", bufs=2))
        ps = ctx.enter_context(tc.tile_pool(name="ps", bufs=2, space="PSUM"))
        psa = ctx.enter_context(tc.tile_pool(name="psa", bufs=1, space="PSUM"))

        # ---- constants ----
        nm_t = cn.tile([128, NJ, I], FP8)
        nc.sync.dma_start(nm_t[:], nm[:].rearrange("(c p) i -> p c i", p=128))
        id8 = cn.tile([128, 128], FP8)
        nc.sync.dma_start(id8[:], identf8[:])
        idr = cn.tile([128, 128], F32R)
        nc.sync.dma_start(idr[:], identr[:])
        ones = cn.tile([128, 512], F32R)
        nc.sync.dma_start(ones[:], onesr[:])
        qs1_t = cn.tile([128, HEADS, IN_DIM // 128, 1], F32R)
        nc.sync.dma_start(qs1_t[:], qs1[:].rearrange("h (c p) o -> p h c o", p=128))
        qs2_t = cn.tile([128, HEADS, H1 // 128, 1], F32R)
        nc.sync.dma_start(qs2_t[:], qs2[:].rearrange("h (c p) o -> p h c o", p=128))
        lin1_t = cn.tile([128, HEADS * H1 // 128, H1], F32R)
        nc.sync.dma_start(lin1_t[:], lin1[:].rearrange("(c p) g -> p c g", p=128))
        res1_t = cn.tile([128, IN_DIM // 128, H1], F32R)
        nc.sync.dma_start(res1_t[:], res1[:].rearrange("(c p) g -> p c g", p=128))
        lin2_t = cn.tile([128, HEADS * H2 // 128, H2], F32R)
        nc.sync.dma_start(lin2_t[:], lin2[:].rearrange("(c p) g -> p c g", p=128))
        res2_t = cn.tile([128, H1 // 128, H2], F32R)
        nc.sync.dma_start(res2_t[:], res2[:].rearrange("(c p) g -> p c g", p=128))

        # ---- persistent big tensors ----
        hmy1 = big.tile([128, IN_DIM // 128, I], F32R, tag="hmy1")
        h2T = big.tile([128, H1 // 128, N], F32R, tag="h2T")
        headsT = big.tile([128, HEADS * H1 // 128, I], F32R, tag="headsT")

        def attention_layer(hT_dram, hT_tile, hmyT, ND, F, WQN, wq_d, qs_t,
                            lin_t, res_t, G):
            """hT_dram: stream the full [128*ND, N] features from DRAM if set,
            else use resident hT_tile [128, ND, N].  hmyT: [128, ND, I].
            Returns hnext [128, G//128, I] f32r."""
            GC, FC, HFC = G // 128, F // 128, HEADS * F // 128
            for h in range(HEADS):
                wqt = wk.tile([128, 4, WQ1], F32R, tag="wqt")
                nc.sync.dma_start(wqt[:, :ND, :WQN],
                                  wq_d[h].rearrange("(c p) n -> p c n", p=128))
                src_ps = ps.tile([1, I], F32, tag="small", bufs=1)
                for dc in range(ND):
                    nc.tensor.matmul(src_ps[:], qs_t[:, h, dc], hmyT[:, dc],
                                     start=(dc == 0), stop=(dc == ND - 1))
                src = wk.tile([1, I], F32R, tag="src")
                nc.vector.tensor_copy(src[:], src_ps[:])

                S = psa.tile([1, I], F32, tag="S")
                U = psa.tile([128, H1 // 128, I], F32, tag="U")

                for jg in range(NJ // 4):
                    if hT_dram is not None:
                        hs = wk.tile([128, ND, 512], F32R, tag="hs", bufs=3)
                        nc.sync.dma_start(
                            hs[:], hT_dram[:, jg * 512:(jg + 1) * 512].rearrange(
                                "(c p) n -> p c n", p=128))
                    for jq in range(4):
                        jc = jg * 4 + jq
                        whq = ps.tile([128, WQN], F32, tag="whq")
                        for dc in range(ND):
                            lhsT = (hs[:, dc, jq * 128:(jq + 1) * 128]
                                    if hT_dram is not None else
                                    hT_tile[:, dc, jc * 128:(jc + 1) * 128])
                            nc.tensor.matmul(whq[:], lhsT, wqt[:, dc, :WQN],
                                             start=(dc == 0), stop=(dc == ND - 1))
                        whsb = wk.tile([128, F], F32R, tag="whsb")
                        nc.vector.tensor_copy(whsb[:], whq[:, :F])
                        dstc = wk.tile([128, 1], F32, tag="dstc")
                        nc.vector.tensor_copy(dstc[:], whq[:, F:F + 1])
                        dstc2 = wk.tile([128, 1], F32, tag="dstc2")
                        nc.vector.tensor_scalar_mul(dstc2[:], dstc[:], 0.2)

                        z = ps.tile([128, I], F32, tag="z")
                        nc.tensor.matmul(z[:], ones[0:1, :128], src[:],
                                         start=True, stop=False)
                        nc.tensor.matmul(z[:], id8[:], nm_t[:, jc],
                                         start=False, stop=True)
                        e1 = wk.tile([128, I], F32, tag="e1")
                        nc.scalar.activation(e1[:], z[:], AF.Exp,
                                             bias=dstc[:], scale=1.0)
                        e2 = wk.tile([128, I], F32, tag="e2")
                        nc.scalar.activation(e2[:], z[:], AF.Exp,
                                             bias=dstc2[:], scale=0.2)
                        v = wk.tile([128, I], F32R, tag="v", bufs=3)
                        nc.vector.tensor_tensor(v[:], e1[:], e2[:], OP.max)

                        nc.tensor.matmul(S[:], ones[:, 0:1], v[:],
                                         start=(jc == 0), stop=(jc == NJ - 1))
                        for fc in range(FC):
                            nc.tensor.matmul(U[:, fc],
                                             whsb[:, fc * 128:(fc + 1) * 128],
                                             v[:], start=(jc == 0),
                                             stop=(jc == NJ - 1))

                s_sb = wk.tile([1, I], F32, tag="srow")
                nc.vector.tensor_copy(s_sb[:], S[:])
                rs = wk.tile([1, I], F32, tag="rsrow")
                nc.vector.reciprocal(rs[:], s_sb[:])
                rsr = wk.tile([1, I], F32R, tag="rsr")
                nc.vector.tensor_copy(rsr[:], rs[:])
                bS = ps.tile([128, I], F32, tag="whq")
                nc.tensor.matmul(bS[:], ones[0:1, :128], rsr[:],
                                 start=True, stop=True)
                bS_sb = wk.tile([128, I], F32, tag="bs", bufs=1)
                nc.scalar.copy(bS_sb[:], bS[:])
                for fc in range(FC):
                    att = wk.tile([128, I], F32, tag="att", bufs=1)
                    nc.vector.tensor_tensor(att[:], U[:, fc], bS_sb[:], OP.mult)
                    tmin = wk.tile([128, I], F32, tag="tmin", bufs=1)
                    nc.vector.tensor_scalar_min(tmin[:], att[:], 0.0)
                    ex = wk.tile([128, I], F32, tag="ex", bufs=1)
                    nc.scalar.activation(ex[:], tmin[:], AF.Exp)
                    rmax = wk.tile([128, I], F32, tag="rmax", bufs=1)
                    nc.vector.tensor_scalar_max(rmax[:], att[:], 0.0)
                    nc.vector.scalar_tensor_tensor(
                        headsT[:, h * FC + fc, :], in0=rmax[:], scalar=-1.0,
                        in1=ex[:], op0=OP.add, op1=OP.add)

            mh = psa.tile([128, H1 // 128, I], F32, tag="U")
            for gc in range(GC):
                for kc in range(HFC):
                    nc.tensor.matmul(mh[:, gc],
                                     lin_t[:, kc, gc * 128:(gc + 1) * 128],
                                     headsT[:, kc, :],
                                     start=(kc == 0), stop=(kc == HFC - 1))
            mh_sb = wk.tile([128, GC, I], F32, tag="mhsb", bufs=1)
            for gc in range(GC):
                tmin = wk.tile([128, I], F32, tag="tmin", bufs=1)
                nc.vector.tensor_scalar_min(tmin[:], mh[:, gc], 0.0)
                ex = wk.tile([128, I], F32, tag="ex", bufs=1)
                nc.scalar.activation(ex[:], tmin[:], AF.Exp)
                rmax = wk.tile([128, I], F32, tag="rmax", bufs=1)
                nc.vector.tensor_scalar_max(rmax[:], mh[:, gc], 0.0)
                nc.vector.scalar_tensor_tensor(
                    mh_sb[:, gc], in0=rmax[:], scalar=-1.0, in1=ex[:],
                    op0=OP.add, op1=OP.add)
            hnext = wk.tile([128, GC, I], F32R, tag="hnext")
            for gc in range(GC):
                rz = ps.tile([128, I], F32, tag="z")
                for kc in range(ND):
                    nc.tensor.matmul(rz[:], res_t[:, kc, gc * 128:(gc + 1) * 128],
                                     hmyT[:, kc], start=(kc == 0),
                                     stop=(kc == ND - 1))
                addt = wk.tile([128, I], F32, tag="att", bufs=1)
                nc.vector.tensor_tensor(addt[:], rz[:], mh_sb[:, gc], OP.add)
                nc.vector.tensor_scalar_max(hnext[:, gc], addt[:], 0.0)
            return hnext

        for enc in ("o", "a"):
            xT_src = xoT if enc == "o" else xaT
            hmy_src = hmyT1_o if enc == "o" else hmyT1_a
            nc.sync.dma_start(hmy1[:], hmy_src[:].rearrange("(c p) n -> p c n", p=128))

            h1n = attention_layer(xT_src, None, hmy1, IN_DIM // 128, H1, WQ1,
                                  wq1, qs1_t, lin1_t, res1_t, H1)
            nc.sync.dma_start(ag1_in[enc][:].rearrange("(c p) i -> p c i", p=128), h1n[:])
            nc.gpsimd.collective_compute("AllGather", OP.bypass, replica_groups=RG,
                                         ins=[ag1_in[enc][:]], outs=[ag1_out[enc][:]])
            for r in range(W):
                nc.sync.dma_start(
                    h2T[:, :, r * I:(r + 1) * I],
                    ag1_out[enc][r * H1:(r + 1) * H1, :].rearrange(
                        "(c p) i -> p c i", p=128))

            h2n = attention_layer(None, h2T, h1n, H1 // 128, H2, WQ2,
                                  wq2, qs2_t, lin2_t, res2_t, H2)
            nc.sync.dma_start(ag2_in[enc][:].rearrange("(c p) i -> p c i", p=128), h2n[:])
            nc.gpsimd.collective_compute("AllGather", OP.bypass, replica_groups=RG,
                                         ins=[ag2_in[enc][:]], outs=[ag2_out[enc][:]])

            if enc == "o":
                x2nat = wk.tile([128, 4, H2], F32, tag="x2nat", bufs=1)
                for o in range(4):
                    tp = ps.tile([128, 128], F32R, tag="z")
                    nc.tensor.transpose(tp[:], h2n[:, 0, o * 128:(o + 1) * 128], idr[:])
                    nc.vector.tensor_copy(x2nat[:, o], tp[:].bitcast(F32))
                nc.sync.dma_start(agn_in[:].rearrange("(c p) f -> p c f", p=128), x2nat[:])
                nc.sync.dma_start(x2slice_d[:].rearrange("(c p) f -> p c f", p=128), x2nat[:])
                nc.gpsimd.collective_compute("AllGather", OP.bypass, replica_groups=RG,
                                             ins=[agn_in[:]], outs=[agn_out[:]])

        # ---------- heads ----------
        sv = {}
        for enc in ("o", "a"):
            acc = wk.tile([128, W], F32, tag="macc", bufs=1)
            for t in range(W):
                xt = wk.tile([128, I], F32, tag="e1")
                nc.sync.dma_start(xt[:],
                                  ag2_out[enc][t * H2:(t + 1) * H2, :].bitcast(F32))
                nc.vector.reduce_sum(acc[:, t:t + 1], xt[:], axis=mybir.AxisListType.X)
            ssum = wk.tile([128, 1], F32, tag="ssum")
            nc.vector.reduce_sum(ssum[:], acc[:], axis=mybir.AxisListType.X)
            svt = wk.tile([128, 1], F32, tag="sv" + enc, bufs=1)
            nc.scalar.activation(svt[:], ssum[:], AF.Sigmoid, scale=1.0 / N)
            sv[enc] = svt

        mw = cn.tile([128, H2], F32)
        nc.sync.dma_start(mw[:], mlp1_w[:])
        mb = cn.tile([1, H2], F32)
        nc.sync.dma_start(mb[:], mlp1_b[:])
        dwT = cn.tile([128, H2], F32)
        nc.sync.dma_start(dwT[:], disc_wT[:])
        dbt = cn.tile([1, 1], F32)
        nc.sync.dma_start(dbt[:], disc_b[:])
        onesf = ones[:].bitcast(F32)

        dc_sb = {}
        for enc in ("o", "a"):
            hos_ps = ps.tile([1, H2], F32, tag="small", bufs=1)
            nc.tensor.matmul(hos_ps[:], sv[enc][:], mw[:], start=True, stop=True)
            cos = wk.tile([1, H2], F32, tag="cos")
            nc.vector.tensor_copy(cos[:], hos_ps[:])
            nc.vector.tensor_tensor(cos[:], cos[:], mb[:], OP.add)
            cT_ps = ps.tile([128, 1], F32, tag="small", bufs=1)
            nc.tensor.matmul(cT_ps[:], cos[:], onesf[0:1, 0:1], start=True, stop=True)
            cT = wk.tile([128, 1], F32, tag="cT")
            nc.vector.tensor_copy(cT[:], cT_ps[:])
            dcp = ps.tile([128, 1], F32, tag="small", bufs=1)
            nc.tensor.matmul(dcp[:], dwT[:], cT[:], start=True, stop=True)
            dct = wk.tile([128, 1], F32, tag="dc" + enc, bufs=1)
            nc.vector.tensor_copy(dct[:], dcp[:])
            dc_sb[enc] = dct

        awt = cn.tile([128, 1], F32)
        nc.sync.dma_start(awt[:], aw1[:])
        abt = cn.tile([1, 1], F32)
        nc.sync.dma_start(abt[:], advb_s[:])

        for enc, rows in (("o", (0, 3)), ("a", (2, 1))):
            same, other = rows
            oth = "a" if enc == "o" else "o"
            arow = 0 if enc == "o" else 1
            for t in range(W):
                xt = wk.tile([128, I], F32, tag="e1")
                nc.sync.dma_start(xt[:],
                                  ag2_out[enc][t * H2:(t + 1) * H2, :].bitcast(F32))
                p1 = ps.tile([1, I], F32, tag="small", bufs=1)
                nc.tensor.matmul(p1[:], dc_sb[enc][:], xt[:], start=True, stop=True)
                r1 = wk.tile([1, I], F32, tag="srow")
                nc.scalar.activation(r1[:], p1[:], AF.Identity, bias=dbt[:])
                nc.sync.dma_start(disc4_d[same:same + 1, t * I:(t + 1) * I], r1[:])
                p2 = ps.tile([1, I], F32, tag="small", bufs=1)
                nc.tensor.matmul(p2[:], dc_sb[oth][:], xt[:], start=True, stop=True)
                r2 = wk.tile([1, I], F32, tag="srow")
                nc.scalar.activation(r2[:], p2[:], AF.Identity, bias=dbt[:])
                nc.sync.dma_start(disc4_d[other:other + 1, t * I:(t + 1) * I], r2[:])
                p3 = ps.tile([1, I], F32, tag="small", bufs=1)
                nc.tensor.matmul(p3[:], awt[:], xt[:], start=True, stop=True)
                r3 = wk.tile([1, I], F32, tag="srow")
                nc.scalar.activation(r3[:], p3[:], AF.Identity, bias=abt[:])
                nc.sync.dma_start(advrows_d[arow:arow + 1, t * I:(t + 1) * I], r3[:])

        # ---------- decoder head ----------
        d1w = cn.tile([128, 2, DEC], F32)
        nc.sync.dma_start(d1w[:], dec1_w[:].rearrange("(c p) n -> p c n", p=128))
        d1b = cn.tile([128, DEC // 128], F32)
        nc.sync.dma_start(d1b[:], dec1_bw[:])
        d2w = cn.tile([128, DEC // 128, 1], F32)
        nc.sync.dma_start(d2w[:], dec2_w[:].rearrange("(c p) o -> p c o", p=128))
        d2b = cn.tile([1, 1], F32)
        nc.sync.dma_start(d2b[:], dec2_b[:])

        nc.sync.dma_start(agn_local[:], agn_out[:])
        e12T = wk.tile([128, 2, I], F32, tag="e12T", bufs=1)
        for k, idx_d in ((0, idx1w), (1, idx2w)):
            idxs = wk.tile([128, I // 16], I16, tag="idxs")
            nc.sync.dma_start(idxs[:], idx_d[:])
            gat = wk.tile([128, I // 128, H2], F32, tag="gat", bufs=1)
            nc.gpsimd.dma_gather(gat[:], agn_local[:], idxs[:], num_idxs=I,
                                 num_idxs_reg=I, elem_size=H2)
            for o in range(I // 128):
                tp = ps.tile([128, 128], F32, tag="z")
                nc.tensor.transpose(tp[:], gat[:, o], idr[:].bitcast(F32))
                nc.vector.tensor_copy(e12T[:, k, o * 128:(o + 1) * 128], tp[:])

        hdec = wk.tile([128, DEC // 128, I], F32, tag="hdec", bufs=1)
        for mc in range(DEC // 128):
            hp = ps.tile([128, I], F32, tag="z")
            for kc in range(2):
                nc.tensor.matmul(hp[:], d1w[:, kc, mc * 128:(mc + 1) * 128],
                                 e12T[:, kc], start=(kc == 0), stop=(kc == 1))
            nc.scalar.activation(hdec[:, mc], hp[:], AF.Relu, bias=d1b[:, mc:mc + 1])
        lp = ps.tile([1, I], F32, tag="small", bufs=1)
        for kc in range(DEC // 128):
            nc.tensor.matmul(lp[:], d2w[:, kc], hdec[:, kc],
                             start=(kc == 0), stop=(kc == DEC // 128 - 1))
        l1row = wk.tile([1, I], F32, tag="srow")
        nc.scalar.activation(l1row[:], lp[:], AF.Identity, bias=d2b[:])
        nc.sync.dma_start(log1s_d[:], l1row[:])
        lrow = wk.tile([1, I], F32, tag="srow")
        nc.scalar.activation(lrow[:], lp[:], AF.Sigmoid, bias=d2b[:])
        nc.sync.dma_start(logs_d[:], lrow[:])

        ctx.close()
    return nc


_cached = {}


def _get_module():
    if "nc" not in _cached:
        nc = _build()
        nc.finalize()
        nc.m = get_hw_module(nc.m)
        _cached["nc"] = nc
    return _cached["nc"]


def _wrap_idx(ix):
    iw = ix.reshape(I // 16, 16).T.astype(np.int16)
    return np.ascontiguousarray(np.tile(iw, (8, 1)))


def prepare_inputs(x_o, x_a, W1, a1, lin1, res1, W2, a2, lin2, res2,
                   mlp1_w, mlp1_b, disc_w, disc_b, dec1_w, dec1_b,
                   dec2_w, dec2_b, adv_w, adv_b, edge_index, idx):
    x_o = np.asarray(x_o, np.float32)
    x_a = np.asarray(x_a, np.float32)
    edge_index = np.asarray(edge_index).astype(np.int64)
    idx = np.asarray(idx).astype(np.int64)
    W1 = np.asarray(W1, np.float32); a1 = np.asarray(a1, np.float32)
    W2 = np.asarray(W2, np.float32); a2 = np.asarray(a2, np.float32)

    mask = np.zeros((N, N), bool)
    mask[edge_index[0], edge_index[1]] = True

    qd1 = np.einsum('hdf,hf->hd', W1, a1[:, H1:])
    qs1v = np.einsum('hdf,hf->hd', W1, a1[:, :H1])
    qd2 = np.einsum('hdf,hf->hd', W2, a2[:, H2:])
    qs2v = np.einsum('hdf,hf->hd', W2, a2[:, :H2])
    wq1v = np.concatenate([W1, qd1[:, :, None],
                           np.zeros((HEADS, IN_DIM, WQ1 - H1 - 1), np.float32)], 2)
    wq2v = np.concatenate([W2, qd2[:, :, None],
                           np.zeros((HEADS, H1, WQ2 - H2 - 1), np.float32)], 2)

    common = {
        "xoT": np.ascontiguousarray(x_o.T), "xaT": np.ascontiguousarray(x_a.T),
        "wq1": np.ascontiguousarray(wq1v), "qs1": np.ascontiguousarray(qs1v[:, :, None]),
        "wq2": np.ascontiguousarray(wq2v), "qs2": np.ascontiguousarray(qs2v[:, :, None]),
        "lin1": np.asarray(lin1, np.float32), "res1": np.asarray(res1, np.float32),
        "lin2": np.asarray(lin2, np.float32), "res2": np.asarray(res2, np.float32),
        "identf8": np.eye(128, dtype=ml_dtypes.float8_e5m2),
        "identr": np.eye(128, dtype=np.float32),
        "onesr": np.ones((128, 512), np.float32),
        "mlp1_w": np.asarray(mlp1_w, np.float32),
        "mlp1_b": np.asarray(mlp1_b, np.float32).reshape(1, H2),
        "disc_wT": np.ascontiguousarray(np.asarray(disc_w, np.float32).T),
        "disc_b": np.asarray(disc_b, np.float32).reshape(1, 1),
        "dec1_w": np.asarray(dec1_w, np.float32),
        "dec1_bw": np.ascontiguousarray(
            np.asarray(dec1_b, np.float32).reshape(DEC // 128, 128).T),
        "dec2_w": np.asarray(dec2_w, np.float32).reshape(DEC, 1),
        "dec2_b": np.asarray(dec2_b, np.float32).reshape(1, 1),
        "aw1": np.ascontiguousarray(np.asarray(adv_w, np.float32).sum(1)[:, None]),
        "advb_s": np.asarray(np.asarray(adv_b, np.float32).sum(),
                             np.float32).reshape(1, 1),
    }
    in_maps = []
    for c in range(W):
        m = dict(common)
        sl = slice(c * I, (c + 1) * I)
        m["nm"] = np.ascontiguousarray(
            np.where(mask[sl, :], 0.0, NEGDEV).astype(np.float32).T
        ).astype(ml_dtypes.float8_e5m2)
        m["hmyT1_o"] = np.ascontiguousarray(x_o[sl].T)
        m["hmyT1_a"] = np.ascontiguousarray(x_a[sl].T)
        m["idx1w"] = _wrap_idx(idx[0, sl])
        m["idx2w"] = _wrap_idx(idx[1, sl])
        in_maps.append(m)
    return in_maps


def assemble(results):
    x2_o = np.concatenate([results[c]["x2slice"] for c in range(W)], 0)
    d4 = results[0]["disc4"]
    ret_os = np.stack([d4[0], d4[1]], 1).astype(np.float32)
    ret_os_a = np.stack([d4[2], d4[3]], 1).astype(np.float32)
    adv = results[0]["advrows"]
    logits = np.concatenate([adv[0], adv[1]])[None, :].astype(np.float32)
    log1 = np.concatenate([results[c]["log1s"][0] for c in range(W)])[:, None]
    log = np.concatenate([results[c]["logs"][0] for c in range(W)])[:, None]
    return (log.astype(np.float32), ret_os, ret_os_a,
            x2_o.astype(np.float32), logits, log1.astype(np.float32))


def kernel(**inputs):
    nc = _get_module()
    in_maps = prepare_inputs(**inputs)
    res = run_bass_kernel_spmd(nc, in_maps, core_ids=list(range(W)))
    return assemble(res.results)


# revision 9
# speedup vs baseline: 1.0653x; 1.0653x over previous
"""Trainium2 Bass kernel for a 2-layer multi-head GAT encoder + heads.

Row-shards the 4096x4096 masked attention across 8 NeuronCores (512 query
rows each), keeps the NxN matrices fused in SBUF (never materialized in
HBM), works in a transposed [feature, node] layout so every contraction
lands on the tensor engine, and AllGathers node features between layers.
"""

import numpy as np
import ml_dtypes

import concourse.bass as bass
import concourse.mybir as mybir
import concourse.tile as tile
from concourse import bacc
from concourse.bass_utils import run_bass_kernel_spmd
from concourse.bass_interp import get_hw_module

F32 = mybir.dt.float32
F32R = mybir.dt.float32r
FP8 = mybir.dt.float8e5
I16 = mybir.dt.int16
AF = mybir.ActivationFunctionType
OP = mybir.AluOpType

N, IN_DIM, H1, H2, HEADS, DEC = 4096, 512, 256, 128, 4, 512
W = 8                 # cores
I = N // W            # queries per core (512)
NJ = N // 128         # 32 j-chunks
NEGDEV = -512.0       # mask add; post-lrelu -102.4, exp() underflows to 0
WQ1 = H1 + 4          # [W1 | qd | pad] -> 260
WQ2 = 2 * H2          # [W2 | qd | pad] -> 256 (fp32r full speed needs >=256)


def _build():
    nc = bacc.Bacc("TRN2", target_bir_lowering=False, debug=False, num_devices=W)

    def inp(name, shape, dt):
        return nc.dram_tensor(name, list(shape), dt, kind="ExternalInput")

    xoT = inp("xoT", [IN_DIM, N], F32R)
    xaT = inp("xaT", [IN_DIM, N], F32R)
    hmyT1_o = inp("hmyT1_o", [IN_DIM, I], F32R)
    hmyT1_a = inp("hmyT1_a", [IN_DIM, I], F32R)
    nm = inp("nm", [N, I], FP8)
    wq1 = inp("wq1", [HEADS, IN_DIM, WQ1], F32R)
    qs1 = inp("qs1", [HEADS, IN_DIM, 1], F32R)
    wq2 = inp("wq2", [HEADS, H1, WQ2], F32R)
    qs2 = inp("qs2", [HEADS, H1, 1], F32R)
    lin1 = inp("lin1", [HEADS * H1, H1], F32R)
    res1 = inp("res1", [IN_DIM, H1], F32R)
    lin2 = inp("lin2", [HEADS * H2, H2], F32R)
    res2 = inp("res2", [H1, H2], F32R)
    identf8 = inp("identf8", [128, 128], FP8)
    identr = inp("identr", [128, 128], F32R)
    onesr = inp("onesr", [128, 512], F32R)
    mlp1_w = inp("mlp1_w", [H2, H2], F32)
    mlp1_b = inp("mlp1_b", [1, H2], F32)
    disc_wT = inp("disc_wT", [H2, H2], F32)
    disc_b = inp("disc_b", [1, 1], F32)
    dec1_w = inp("dec1_w", [2 * H2, DEC], F32)
    dec1_bw = inp("dec1_bw", [128, DEC // 128], F32)
    dec2_w = inp("dec2_w", [DEC, 1], F32)
    dec2_b = inp("dec2_b", [1, 1], F32)
    aw1 = inp("aw1", [H2, 1], F32)
    advb_s = inp("advb_s", [1, 1], F32)
    idx1w = inp("idx1w", [128, I // 16], I16)
    idx2w = inp("idx2w", [128, I // 16], I16)

    x2slice_d = nc.dram_tensor("x2slice", [I, H2], F32, kind="ExternalOutput")
    disc4_d = nc.dram_tensor("disc4", [4, N], F32, kind="ExternalOutput")
    advrows_d = nc.dram_tensor("advrows", [2, N], F32, kind="ExternalOutput")
    log1s_d = nc.dram_tensor("log1s", [1, I], F32, kind="ExternalOutput")
    logs_d = nc.dram_tensor("logs", [1, I], F32, kind="ExternalOutput")

    ag1_in, ag1_out, ag2_in, ag2_out = {}, {}, {}, {}
    for e in ("o", "a"):
        ag1_in[e] = nc.dram_tensor(f"ag1_in_{e}", [H1, I], F32R)
        ag1_out[e] = nc.dram_tensor(f"ag1_out_{e}", [W * H1, I], F32R, addr_space="Shared")
        ag2_in[e] = nc.dram_tensor(f"ag2_in_{e}", [H2, I], F32R)
        ag2_out[e] = nc.dram_tensor(f"ag2_out_{e}", [W * H2, I], F32R, addr_space="Shared")
    agn_in = nc.dram_tensor("agn_in", [I, H2], F32)
    agn_out = nc.dram_tensor("agn_out", [N, H2], F32, addr_space="Shared")
    agn_local = nc.dram_tensor("agn_local", [N, H2], F32)
    RG = [list(range(W))]

    with tile.TileContext(nc) as tc:
        import contextlib
        ctx = contextlib.ExitStack()
        cn = ctx.enter_context(tc.tile_pool(name="cn", bufs=1))
        big = ctx.enter_context(tc.tile_pool(name="big", bufs=1))
        wk = ctx.enter_context(tc.tile_pool(name="wk", bufs=2))
        ps = ctx.enter_context(tc.tile_pool(name="ps", bufs=2, space="PSUM"))
        psa = ctx.enter_context(tc.tile_pool(name="psa", bufs=1, space="PSUM"))

        # ---- constants ----
        nm_t = cn.tile([128, NJ, I], FP8)
        nc.sync.dma_start(nm_t[:], nm[:].rearrange("(c p) i -> p c i", p=128))
        id8 = cn.tile([128, 128], FP8)
        nc.sync.dma_start(id8[:], identf8[:])
        idr = cn.tile([128, 128], F32R)
        nc.sync.dma_start(idr[:], identr[:])
        ones = cn.tile([128, 512], F32R)
        nc.sync.dma_start(ones[:], onesr[:])
        qs1_t = cn.tile([128, HEADS, IN_DIM // 128, 1], F32R)
        nc.sync.dma_start(qs1_t[:], qs1[:].rearrange("h (c p) o -> p h c o", p=128))
        qs2_t = cn.tile([128, HEADS, H1 // 128, 1], F32R)
        nc.sync.dma_start(qs2_t[:], qs2[:].rearrange("h (c p) o -> p h c o", p=128))
        lin1_t = cn.tile([128, HEADS * H1 // 128, H1], F32R)
        nc.sync.dma_start(lin1_t[:], lin1[:].rearrange("(c p) g -> p c g", p=128))
        res1_t = cn.tile([128, IN_DIM // 128, H1], F32R)
        nc.sync.dma_start(res1_t[:], res1[:].rearrange("(c p) g -> p c g", p=128))
        lin2_t = cn.tile([128, HEADS * H2 // 128, H2], F32R)
        nc.sync.dma_start(lin2_t[:], lin2[:].rearrange("(c p) g -> p c g", p=128))
        res2_t = cn.tile([128, H1 // 128, H2], F32R)
        nc.sync.dma_start(res2_t[:], res2[:].rearrange("(c p) g -> p c g", p=128))

        # ---- persistent big tensors ----
        hmy1 = big.tile([128, IN_DIM // 128, I], F32R, tag="hmy1")
        h2T = big.tile([128, H1 // 128, N], F32R, tag="h2T")
        headsT = big.tile([128, HEADS * H1 // 128, I], F32R, tag="headsT")

        def attention_layer(hT_dram, hT_tile, hmyT, ND, F, WQN, wq_d, qs_t,
                            lin_t, res_t, G):
            """hT_dram: stream the full [128*ND, N] features from DRAM if set,
            else use resident hT_tile [128, ND, N].  hmyT: [128, ND, I].
            Returns hnext [128, G//128, I] f32r."""
            GC, FC, HFC = G // 128, F // 128, HEADS * F // 128
            for h in range(HEADS):
                wqt = wk.tile([128, 4, WQ1], F32R, tag="wqt")
                nc.sync.dma_start(wqt[:, :ND, :WQN],
                                  wq_d[h].rearrange("(c p) n -> p c n", p=128))
                src_ps = ps.tile([1, I], F32, tag="small", bufs=1)
                for dc in range(ND):
                    nc.tensor.matmul(src_ps[:], qs_t[:, h, dc], hmyT[:, dc],
                                     start=(dc == 0), stop=(dc == ND - 1))
                src = wk.tile([1, I], F32R, tag="src")
                nc.vector.tensor_copy(src[:], src_ps[:])

                S = psa.tile([1, I], F32, tag="S")
                U = psa.tile([128, H1 // 128, I], F32, tag="U")

                for jg in range(NJ // 4):
                    if hT_dram is not None:
                        hs = wk.tile([128, ND, 512], F32R, tag="hs", bufs=3)
                        nc.sync.dma_start(
                            hs[:], hT_dram[:, jg * 512:(jg + 1) * 512].rearrange(
                                "(c p) n -> p c n", p=128))
                    for jq in range(4):
                        jc = jg * 4 + jq
                        whq = ps.tile([128, WQN], F32, tag="whq")
                        for dc in range(ND):
                            lhsT = (hs[:, dc, jq * 128:(jq + 1) * 128]
                                    if hT_dram is not None else
                                    hT_tile[:, dc, jc * 128:(jc + 1) * 128])
                            nc.tensor.matmul(whq[:], lhsT, wqt[:, dc, :WQN],
                                             start=(dc == 0), stop=(dc == ND - 1))
                        whsb = wk.tile([128, F], F32R, tag="whsb", bufs=3)
                        nc.vector.tensor_copy(whsb[:], whq[:, :F])
                        dstc = wk.tile([128, 2], F32, tag="dstc", bufs=3)
                        nc.vector.tensor_copy(dstc[:], whq[:, F:F + 2])

                        z = ps.tile([128, I], F32, tag="z")
                        nc.tensor.matmul(z[:], ones[0:1, :128], src[:],
                                         start=True, stop=False)
                        nc.tensor.matmul(z[:], id8[:], nm_t[:, jc],
                                         start=False, stop=True)
                        e1 = wk.tile([128, I], F32, tag="e1", bufs=3)
                        nc.scalar.activation(e1[:], z[:], AF.Exp,
                                             bias=dstc[:, 0:1], scale=1.0)
                        e2 = wk.tile([128, I], F32, tag="e2", bufs=3)
                        nc.scalar.activation(e2[:], z[:], AF.Exp,
                                             bias=dstc[:, 1:2], scale=0.2)
                        v = wk.tile([128, I], F32R, tag="v", bufs=3)
                        nc.vector.tensor_tensor(v[:], e1[:], e2[:], OP.max)

                        nc.tensor.matmul(S[:], ones[:, 0:1], v[:],
                                         start=(jc == 0), stop=(jc == NJ - 1))
                        for fc in range(FC):
                            nc.tensor.matmul(U[:, fc],
                                             whsb[:, fc * 128:(fc + 1) * 128],
                                             v[:], start=(jc == 0),
                                             stop=(jc == NJ - 1))

                s_sb = wk.tile([1, I], F32, tag="srow")
                nc.vector.tensor_copy(s_sb[:], S[:])
                rs = wk.tile([1, I], F32, tag="rsrow")
                nc.vector.reciprocal(rs[:], s_sb[:])
                rsr = wk.tile([1, I], F32R, tag="rsr")
                nc.vector.tensor_copy(rsr[:], rs[:])
                bS = ps.tile([128, I], F32, tag="whq")
                nc.tensor.matmul(bS[:], ones[0:1, :128], rsr[:],
                                 start=True, stop=True)
                bS_sb = wk.tile([128, I], F32, tag="bs", bufs=1)
                nc.scalar.copy(bS_sb[:], bS[:])
                for fc in range(FC):
                    att = wk.tile([128, I], F32, tag="att", bufs=1)
                    nc.vector.tensor_tensor(att[:], U[:, fc], bS_sb[:], OP.mult)
                    tmin = wk.tile([128, I], F32, tag="tmin", bufs=1)
                    nc.vector.tensor_scalar_min(tmin[:], att[:], 0.0)
                    ex = wk.tile([128, I], F32, tag="ex", bufs=1)
                    nc.scalar.activation(ex[:], tmin[:], AF.Exp)
                    rmax = wk.tile([128, I], F32, tag="rmax", bufs=1)
                    nc.vector.tensor_scalar_max(rmax[:], att[:], 0.0)
                    nc.vector.scalar_tensor_tensor(
                        headsT[:, h * FC + fc, :], in0=rmax[:], scalar=-1.0,
                        in1=ex[:], op0=OP.add, op1=OP.add)

            mh = psa.tile([128, H1 // 128, I], F32, tag="U")
            for gc in range(GC):
                for kc in range(HFC):
                    nc.tensor.matmul(mh[:, gc],
                                     lin_t[:, kc, gc * 128:(gc + 1) * 128],
                                     headsT[:, kc, :],
                                     start=(kc == 0), stop=(kc == HFC - 1))
            mh_sb = wk.tile([128, GC, I], F32, tag="mhsb", bufs=1)
            for gc in range(GC):
                tmin = wk.tile([128, I], F32, tag="tmin", bufs=1)
                nc.vector.tensor_scalar_min(tmin[:], mh[:, gc], 0.0)
                ex = wk.tile([128, I], F32, tag="ex", bufs=1)
                nc.scalar.activation(ex[:], tmin[:], AF.Exp)
                rmax = wk.tile([128, I], F32, tag="rmax", bufs=1)
                nc.vector.tensor_scalar_max(rmax[:], mh[:, gc], 0.0)
                nc.vector.scalar_tensor_tensor(
                    mh_sb[:, gc], in0=rmax[:], scalar=-1.0, in1=ex[:],
                    op0=OP.add, op1=OP.add)
            hnext = wk.tile([128, GC, I], F32R, tag="hnext")
            for gc in range(GC):
                rz = ps.tile([128, I], F32, tag="z")
                for kc in range(ND):
                    nc.tensor.matmul(rz[:], res_t[:, kc, gc * 128:(gc + 1) * 128],
                                     hmyT[:, kc], start=(kc == 0),
                                     stop=(kc == ND - 1))
                addt = wk.tile([128, I], F32, tag="att", bufs=1)
                nc.vector.tensor_tensor(addt[:], rz[:], mh_sb[:, gc], OP.add)
                nc.vector.tensor_scalar_max(hnext[:, gc], addt[:], 0.0)
            return hnext

        for enc in ("o", "a"):
            xT_src = xoT if enc == "o" else xaT
            hmy_src = hmyT1_o if enc == "o" else hmyT1_a
            nc.sync.dma_start(hmy1[:], hmy_src[:].rearrange("(c p) n -> p c n", p=128))

            h1n = attention_layer(xT_src, None, hmy1, IN_DIM // 128, H1, WQ1,
                                  wq1, qs1_t, lin1_t, res1_t, H1)
            nc.sync.dma_start(ag1_in[enc][:].rearrange("(c p) i -> p c i", p=128), h1n[:])
            nc.gpsimd.collective_compute("AllGather", OP.bypass, replica_groups=RG,
                                         ins=[ag1_in[enc][:]], outs=[ag1_out[enc][:]])
            for r in range(W):
                nc.sync.dma_start(
                    h2T[:, :, r * I:(r + 1) * I],
                    ag1_out[enc][r * H1:(r + 1) * H1, :].rearrange(
                        "(c p) i -> p c i", p=128))

            h2n = attention_layer(None, h2T, h1n, H1 // 128, H2, WQ2,
                                  wq2, qs2_t, lin2_t, res2_t, H2)
            nc.sync.dma_start(ag2_in[enc][:].rearrange("(c p) i -> p c i", p=128), h2n[:])
            nc.gpsimd.collective_compute("AllGather", OP.bypass, replica_groups=RG,
                                         ins=[ag2_in[enc][:]], outs=[ag2_out[enc][:]])

            if enc == "o":
                x2nat = wk.tile([128, 4, H2], F32, tag="x2nat", bufs=1)
                for o in range(4):
                    tp = ps.tile([128, 128], F32R, tag="z")
                    nc.tensor.transpose(tp[:], h2n[:, 0, o * 128:(o + 1) * 128], idr[:])
                    nc.vector.tensor_copy(x2nat[:, o], tp[:].bitcast(F32))
                nc.sync.dma_start(agn_in[:].rearrange("(c p) f -> p c f", p=128), x2nat[:])
                nc.sync.dma_start(x2slice_d[:].rearrange("(c p) f -> p c f", p=128), x2nat[:])
                agn_cc = nc.gpsimd.collective_compute(
                    "AllGather", OP.bypass, replica_groups=RG,
                    ins=[agn_in[:]], outs=[agn_out[:]])

        # ---------- heads ----------
        sv = {}
        for enc in ("o", "a"):
            acc = wk.tile([128, W], F32, tag="macc", bufs=1)
            for t in range(W):
                xt = wk.tile([128, I], F32, tag="e1", bufs=3)
                nc.sync.dma_start(xt[:],
                                  ag2_out[enc][t * H2:(t + 1) * H2, :].bitcast(F32))
                nc.vector.reduce_sum(acc[:, t:t + 1], xt[:], axis=mybir.AxisListType.X)
            ssum = wk.tile([128, 1], F32, tag="ssum")
            nc.vector.reduce_sum(ssum[:], acc[:], axis=mybir.AxisListType.X)
            svt = wk.tile([128, 1], F32, tag="sv" + enc, bufs=1)
            nc.scalar.activation(svt[:], ssum[:], AF.Sigmoid, scale=1.0 / N)
            sv[enc] = svt

        mw = cn.tile([128, H2], F32)
        nc.sync.dma_start(mw[:], mlp1_w[:])
        mb = cn.tile([1, H2], F32)
        nc.sync.dma_start(mb[:], mlp1_b[:])
        dwT = cn.tile([128, H2], F32)
        nc.sync.dma_start(dwT[:], disc_wT[:])
        dbt = cn.tile([1, 1], F32)
        nc.sync.dma_start(dbt[:], disc_b[:])
        onesf = ones[:].bitcast(F32)

        dc_sb = {}
        for enc in ("o", "a"):
            hos_ps = ps.tile([1, H2], F32, tag="small", bufs=1)
            nc.tensor.matmul(hos_ps[:], sv[enc][:], mw[:], start=True, stop=True)
            cos = wk.tile([1, H2], F32, tag="cos")
            nc.vector.tensor_copy(cos[:], hos_ps[:])
            nc.vector.tensor_tensor(cos[:], cos[:], mb[:], OP.add)
            cT_ps = ps.tile([128, 1], F32, tag="small", bufs=1)
            nc.tensor.matmul(cT_ps[:], cos[:], onesf[0:1, 0:1], start=True, stop=True)
            cT = wk.tile([128, 1], F32, tag="cT")
            nc.vector.tensor_copy(cT[:], cT_ps[:])
            dcp = ps.tile([128, 1], F32, tag="small", bufs=1)
            nc.tensor.matmul(dcp[:], dwT[:], cT[:], start=True, stop=True)
            dct = wk.tile([128, 1], F32, tag="dc" + enc, bufs=1)
            nc.vector.tensor_copy(dct[:], dcp[:])
            dc_sb[enc] = dct

        awt = cn.tile([128, 1], F32)
        nc.sync.dma_start(awt[:], aw1[:])
        abt = cn.tile([1, 1], F32)
        nc.sync.dma_start(abt[:], advb_s[:])

        for enc, rows in (("o", (0, 3)), ("a", (2, 1))):
            same, other = rows
            oth = "a" if enc == "o" else "o"
            arow = 0 if enc == "o" else 1
            for t in range(W):
                xt = wk.tile([128, I], F32, tag="e1", bufs=3)
                nc.sync.dma_start(xt[:],
                                  ag2_out[enc][t * H2:(t + 1) * H2, :].bitcast(F32))
                p1 = ps.tile([1, I], F32, tag="small", bufs=1)
                nc.tensor.matmul(p1[:], dc_sb[enc][:], xt[:], start=True, stop=True)
                r1 = wk.tile([1, I], F32, tag="srow")
                nc.scalar.activation(r1[:], p1[:], AF.Identity, bias=dbt[:])
                nc.sync.dma_start(disc4_d[same:same + 1, t * I:(t + 1) * I], r1[:])
                p2 = ps.tile([1, I], F32, tag="small", bufs=1)
                nc.tensor.matmul(p2[:], dc_sb[oth][:], xt[:], start=True, stop=True)
                r2 = wk.tile([1, I], F32, tag="srow")
                nc.scalar.activation(r2[:], p2[:], AF.Identity, bias=dbt[:])
                nc.sync.dma_start(disc4_d[other:other + 1, t * I:(t + 1) * I], r2[:])
                p3 = ps.tile([1, I], F32, tag="small", bufs=1)
                nc.tensor.matmul(p3[:], awt[:], xt[:], start=True, stop=True)
                r3 = wk.tile([1, I], F32, tag="srow")
                nc.scalar.activation(r3[:], p3[:], AF.Identity, bias=abt[:])
                nc.sync.dma_start(advrows_d[arow:arow + 1, t * I:(t + 1) * I], r3[:])

        # ---------- decoder head ----------
        d1w = cn.tile([128, 2, DEC], F32)
        nc.sync.dma_start(d1w[:], dec1_w[:].rearrange("(c p) n -> p c n", p=128))
        d1b = cn.tile([128, DEC // 128], F32)
        nc.sync.dma_start(d1b[:], dec1_bw[:])
        d2w = cn.tile([128, DEC // 128, 1], F32)
        nc.sync.dma_start(d2w[:], dec2_w[:].rearrange("(c p) o -> p c o", p=128))
        d2b = cn.tile([1, 1], F32)
        nc.sync.dma_start(d2b[:], dec2_b[:])

        from concourse.tile import add_dep_helper
        agn_cp = nc.sync.dma_start(agn_local[:], agn_out[:])
        add_dep_helper(agn_cp.ins, agn_cc.ins, sync=True,
                       reason="agn_local copy waits on natural-x2 AllGather")
        e12T = wk.tile([128, 2, I], F32, tag="e12T", bufs=1)
        for k, idx_d in ((0, idx1w), (1, idx2w)):
            idxs = wk.tile([128, I // 16], I16, tag="idxs")
            nc.sync.dma_start(idxs[:], idx_d[:])
            gat = wk.tile([128, I // 128, H2], F32, tag="gat", bufs=1)
            g_inst = nc.gpsimd.dma_gather(gat[:], agn_local[:], idxs[:], num_idxs=I,
                                          num_idxs_reg=I, elem_size=H2)
            add_dep_helper(g_inst.ins, agn_cp.ins, sync=True,
                           reason="entity gather waits on agn_local copy")
            for o in range(I // 128):
                tp = ps.tile([128, 128], F32, tag="z")
                nc.tensor.transpose(tp[:], gat[:, o], idr[:].bitcast(F32))
                nc.vector.tensor_copy(e12T[:, k, o * 128:(o + 1) * 128], tp[:])

        hdec = wk.tile([128, DEC // 128, I], F32, tag="hdec", bufs=1)
        for mc in range(DEC // 128):
            hp = ps.tile([128, I], F32, tag="z")
            for kc in range(2):
                nc.tensor.matmul(hp[:], d1w[:, kc, mc * 128:(mc + 1) * 128],
                                 e12T[:, kc], start=(kc == 0), stop=(kc == 1))
            nc.scalar.activation(hdec[:, mc], hp[:], AF.Relu, bias=d1b[:, mc:mc + 1])
        lp = ps.tile([1, I], F32, tag="small", bufs=1)
        for kc in range(DEC // 128):
            nc.tensor.matmul(lp[:], d2w[:, kc], hdec[:, kc],
                             start=(kc == 0), stop=(kc == DEC // 128 - 1))
        l1row = wk.tile([1, I], F32, tag="srow")
        nc.scalar.activation(l1row[:], lp[:], AF.Identity, bias=d2b[:])
        nc.sync.dma_start(log1s_d[:], l1row[:])
        lrow = wk.tile([1, I], F32, tag="srow")
        nc.scalar.activation(lrow[:], lp[:], AF.Sigmoid, bias=d2b[:])
        nc.sync.dma_start(logs_d[:], lrow[:])

        ctx.close()
    return nc


_cached = {}


def _get_module():
    if "nc" not in _cached:
        nc = _build()
        nc.finalize()
        nc.m = get_hw_module(nc.m)
        _cached["nc"] = nc
    return _cached["nc"]


def _wrap_idx(ix):
    iw = ix.reshape(I // 16, 16).T.astype(np.int16)
    return np.ascontiguousarray(np.tile(iw, (8, 1)))


def prepare_inputs(x_o, x_a, W1, a1, lin1, res1, W2, a2, lin2, res2,
                   mlp1_w, mlp1_b, disc_w, disc_b, dec1_w, dec1_b,
                   dec2_w, dec2_b, adv_w, adv_b, edge_index, idx):
    x_o = np.asarray(x_o, np.float32)
    x_a = np.asarray(x_a, np.float32)
    edge_index = np.asarray(edge_index).astype(np.int64)
    idx = np.asarray(idx).astype(np.int64)
    W1 = np.asarray(W1, np.float32); a1 = np.asarray(a1, np.float32)
    W2 = np.asarray(W2, np.float32); a2 = np.asarray(a2, np.float32)

    mask = np.zeros((N, N), bool)
    mask[edge_index[0], edge_index[1]] = True

    qd1 = np.einsum('hdf,hf->hd', W1, a1[:, H1:])
    qs1v = np.einsum('hdf,hf->hd', W1, a1[:, :H1])
    qd2 = np.einsum('hdf,hf->hd', W2, a2[:, H2:])
    qs2v = np.einsum('hdf,hf->hd', W2, a2[:, :H2])
    wq1v = np.concatenate([W1, qd1[:, :, None], 0.2 * qd1[:, :, None],
                           np.zeros((HEADS, IN_DIM, WQ1 - H1 - 2), np.float32)], 2)
    wq2v = np.concatenate([W2, qd2[:, :, None], 0.2 * qd2[:, :, None],
                           np.zeros((HEADS, H1, WQ2 - H2 - 2), np.float32)], 2)

    common = {
        "xoT": np.ascontiguousarray(x_o.T), "xaT": np.ascontiguousarray(x_a.T),
        "wq1": np.ascontiguousarray(wq1v), "qs1": np.ascontiguousarray(qs1v[:, :, None]),
        "wq2": np.ascontiguousarray(wq2v), "qs2": np.ascontiguousarray(qs2v[:, :, None]),
        "lin1": np.asarray(lin1, np.float32), "res1": np.asarray(res1, np.float32),
        "lin2": np.asarray(lin2, np.float32), "res2": np.asarray(res2, np.float32),
        "identf8": np.eye(128, dtype=ml_dtypes.float8_e5m2),
        "identr": np.eye(128, dtype=np.float32),
        "onesr": np.ones((128, 512), np.float32),
        "mlp1_w": np.asarray(mlp1_w, np.float32),
        "mlp1_b": np.asarray(mlp1_b, np.float32).reshape(1, H2),
        "disc_wT": np.ascontiguousarray(np.asarray(disc_w, np.float32).T),
        "disc_b": np.asarray(disc_b, np.float32).reshape(1, 1),
        "dec1_w": np.asarray(dec1_w, np.float32),
        "dec1_bw": np.ascontiguousarray(
            np.asarray(dec1_b, np.float32).reshape(DEC // 128, 128).T),
        "dec2_w": np.asarray(dec2_w, np.float32).reshape(DEC, 1),
        "dec2_b": np.asarray(dec2_b, np.float32).reshape(1, 1),
        "aw1": np.ascontiguousarray(np.asarray(adv_w, np.float32).sum(1)[:, None]),
        "advb_s": np.asarray(np.asarray(adv_b, np.float32).sum(),
                             np.float32).reshape(1, 1),
    }
    in_maps = []
    for c in range(W):
        m = dict(common)
        sl = slice(c * I, (c + 1) * I)
        m["nm"] = np.ascontiguousarray(
            np.where(mask[sl, :], 0.0, NEGDEV).astype(np.float32).T
        ).astype(ml_dtypes.float8_e5m2)
        m["hmyT1_o"] = np.ascontiguousarray(x_o[sl].T)
        m["hmyT1_a"] = np.ascontiguousarray(x_a[sl].T)
        m["idx1w"] = _wrap_idx(idx[0, sl])
        m["idx2w"] = _wrap_idx(idx[1, sl])
        in_maps.append(m)
    return in_maps


def assemble(results):
    x2_o = np.concatenate([results[c]["x2slice"] for c in range(W)], 0)
    d4 = results[0]["disc4"]
    ret_os = np.stack([d4[0], d4[1]], 1).astype(np.float32)
    ret_os_a = np.stack([d4[2], d4[3]], 1).astype(np.float32)
    adv = results[0]["advrows"]
    logits = np.concatenate([adv[0], adv[1]])[None, :].astype(np.float32)
    log1 = np.concatenate([results[c]["log1s"][0] for c in range(W)])[:, None]
    log = np.concatenate([results[c]["logs"][0] for c in range(W)])[:, None]
    return (log.astype(np.float32), ret_os, ret_os_a,
            x2_o.astype(np.float32), logits, log1.astype(np.float32))


def kernel(**inputs):
    nc = _get_module()
    in_maps = prepare_inputs(**inputs)
    res = run_bass_kernel_spmd(nc, in_maps, core_ids=list(range(W)))
    return assemble(res.results)
